# revision 13
# baseline (speedup 1.0000x reference)
"""CAM (channel attention) module kernel for 8 TRN2 NeuronCores.

Reference computation (per batch b of 32, C=2048, N=H*W=196):
    E = q @ q.T                      # [C, C] channel energy
    A = softmax(rowmax(E) - E)       # == softmax(-E) row-wise (shift cancels)
    out = gamma * (A @ q) + x
    y = conv1x1(out, W) + bias       # 2048 -> 512

Sharding: pure data-parallel over batch, 4 batches per core, conv weights
replicated. No collectives.

Per-core kernel design (V2, symmetric-E):
  - E = q q^T is symmetric: compute only block-upper-triangle (row-tile j
    covers columns [256*(j//2), 2048) in 256-wide fp32r matmuls).
  - U = exp(-E - 40) via ScalarE activation straight out of PSUM into bf16
    (constant shift keeps exp in fp32 range for N(0,1) inputs of this size;
    any constant cancels in the softmax normalization).
  - Lower-triangle U blocks are mirror-filled with SBUF->SBUF DMA
    transposes (bf16 XBAR path, off-engine).
  - U symmetric => stored U tiles serve directly as lhsT for O = U @ q
    (bf16, free dim 197: q plus a ones column that yields the row sums Z).
  - normalize + residual: xout = O[:, :196] * (gamma/Z) + x on VectorE.
  - 1x1 conv as matmul over C with batch-pair-concatenated free dim (392)
    in float32r, bias added on the PSUM->SBUF evacuation.
"""

import numpy as np

B = 32
NCORES = 8
BL = B // NCORES  # batches per core
C = 2048
HW = 196
OUT = 512
CT = C // 128  # 16 c-tiles
SHIFT = 40.0

_CACHE = {}


def _build_nc(reps=1):
    import contextlib
    import concourse.bacc as bacc
    import concourse.tile as tile
    import concourse.mybir as mybir

    f32 = mybir.dt.float32
    f32r = mybir.dt.float32r
    bf16 = mybir.dt.bfloat16
    FT = mybir.ActivationFunctionType
    ALU = mybir.AluOpType

    nc = bacc.Bacc("TRN2", target_bir_lowering=False, debug=False,
                   num_devices=NCORES)

    qTd = nc.dram_tensor("qT", [BL, 2, 128, C], f32r, kind="ExternalInput")
    qnd = nc.dram_tensor("qn", [BL, CT, 128, HW], f32, kind="ExternalInput")
    qbd = nc.dram_tensor("qb", [BL, CT, 128, HW + 1], bf16,
                         kind="ExternalInput")
    wTd = nc.dram_tensor("wT", [CT, 128, OUT], f32r, kind="ExternalInput")
    biasd = nc.dram_tensor("bias", [128, OUT // 128], f32, kind="ExternalInput")
    gammad = nc.dram_tensor("gammac", [128, 1], f32, kind="ExternalInput")
    outd = nc.dram_tensor("out", [BL, OUT, HW], f32, kind="ExternalOutput")

    with tile.TileContext(nc) as tc:
        with (
            tc.tile_pool(name="const", bufs=1) as constp,
            tc.tile_pool(name="qt", bufs=2) as qtp,
            tc.tile_pool(name="qbp", bufs=2) as qbp,
            tc.tile_pool(name="qnp", bufs=2) as qnp,
            tc.tile_pool(name="u", bufs=1) as up,
            tc.tile_pool(name="xo", bufs=1) as xop,
            tc.tile_pool(name="y", bufs=4) as yp,
            tc.tile_pool(name="z", bufs=8) as zp,
            tc.tile_pool(name="psum", bufs=2, space="PSUM") as psp,
        ):
            wT_s = constp.tile([128, CT, OUT], f32r)
            nc.sync.dma_start(wT_s[:], wTd[:].rearrange("i p o -> p i o"))
            bias_s = constp.tile([128, OUT // 128], f32)
            nc.sync.dma_start(bias_s[:], biasd[:])
            gamma_s = constp.tile([128, 1], f32)
            nc.sync.dma_start(gamma_s[:], gammad[:])
            shift_s = constp.tile([128, 1], f32)
            nc.vector.memset(shift_s[:], -SHIFT)

            U = up.tile([128, CT, C], bf16)

            rep_ctx = (
                tc.For_i(0, reps, 1,
                         hint_engines=tuple(mybir.EngineType))
                if reps > 1 else contextlib.nullcontext()
            )
            with rep_ctx:
                xo = None
                for b in range(BL):
                    qT_s = qtp.tile([128, 2, C], f32r, tag="qt")
                    nc.sync.dma_start(qT_s[:], qTd[b].rearrange("k p c -> p k c"))
                    qb_s = qbp.tile([128, CT, HW + 1], bf16, tag="qb")
                    nc.sync.dma_start(qb_s[:], qbd[b].rearrange("i p n -> p i n"))
                    qn_s = qnp.tile([128, CT, HW], f32, tag="qn")
                    nc.sync.dma_start(qn_s[:], qnd[b].rearrange("i p n -> p i n"))

                    # ---- upper-triangle E (fp32r, 256-col chunks) + exp ----
                    for j in range(CT):
                        dstart = 256 * (j // 2)
                        pe = psp.tile([128, C], f32, tag="ps")
                        for m in range(dstart, C, 256):
                            for k in range(2):
                                nc.tensor.matmul(
                                    pe[:, m:m + 256],
                                    qT_s[:, k, 128 * j:128 * (j + 1)],
                                    qT_s[:, k, m:m + 256],
                                    start=(k == 0),
                                    stop=(k == 1),
                                )
                        nc.scalar.activation(
                            U[:, j, dstart:], pe[:, dstart:], FT.Exp,
                            bias=shift_s[:], scale=-1.0,
                        )

                    # ---- mirror lower-triangle blocks: one batched
                    # block-transpose DMA per source row-tile t writes
                    # U[:, j, 128t:128(t+1)] = U[t-tile block j]^T for all
                    # j >= j0(t) (3D-out XBAR transpose semantics).
                    for t in range(CT):
                        j0 = 2 * (t // 2) + 2
                        if j0 >= CT:
                            continue
                        nc.scalar.dma_start_transpose(
                            U[:, j0:CT, 128 * t:128 * (t + 1)],
                            U[:, t, 128 * j0:C],
                        )

                    if b % 2 == 0:
                        xo = xop.tile([128, CT, 2 * HW], f32r, tag="xo")
                    off = (b % 2) * HW

                    # ---- O|Z = U @ [q|1]; high j first (they need no mirrors)
                    for j in range(CT - 1, -1, -1):
                        po = psp.tile([128, C], f32, tag="ps")
                        for i in range(CT):
                            nc.tensor.matmul(
                                po[:, :HW + 1],
                                U[:, i, 128 * j:128 * (j + 1)],
                                qb_s[:, i, :],
                                start=(i == 0),
                                stop=(i == CT - 1),
                            )
                        rg = zp.tile([128, 2], f32, tag="rg")
                        nc.vector.reciprocal(rg[:, 0:1], po[:, HW:HW + 1])
                        nc.vector.tensor_tensor(
                            rg[:, 1:2], rg[:, 0:1], gamma_s[:], ALU.mult)
                        nc.vector.tensor_scalar_mul(
                            xo[:, j, off:off + HW], po[:, :HW], rg[:, 1:2])
                        nc.vector.tensor_tensor(
                            xo[:, j, off:off + HW], xo[:, j, off:off + HW],
                            qn_s[:, j, :], ALU.add)

                    # ---- 1x1 conv on a pair of batches (free dim 392) ----
                    if b % 2 == 1:
                        b0 = b - 1
                        for t in range(OUT // 128):
                            pc = psp.tile([128, C], f32, tag="ps")
                            for i in range(CT):
                                nc.tensor.matmul(
                                    pc[:, :2 * HW],
                                    wT_s[:, i, 128 * t:128 * (t + 1)],
                                    xo[:, i, :],
                                    start=(i == 0),
                                    stop=(i == CT - 1),
                                )
                            y = yp.tile([128, 2 * HW], f32, tag="y")
                            nc.vector.tensor_scalar_add(
                                y[:], pc[:, :2 * HW], bias_s[:, t:t + 1])
                            nc.sync.dma_start(
                                outd[b0, 128 * t:128 * (t + 1), :], y[:, :HW])
                            nc.sync.dma_start(
                                outd[b, 128 * t:128 * (t + 1), :],
                                y[:, HW:2 * HW])

    nc.compile()
    return nc


def _get_nc():
    if "nc" not in _CACHE:
        _CACHE["nc"] = _build_nc()
    return _CACHE["nc"]


def _prep_in_maps(x, gamma, conv_w, conv_b):
    import ml_dtypes

    x = np.ascontiguousarray(np.asarray(x, dtype=np.float32))
    q = x.reshape(B, C, HW)
    W2 = np.asarray(conv_w, dtype=np.float32).reshape(OUT, C)
    wT = np.ascontiguousarray(W2.T).reshape(CT, 128, OUT)
    bias = np.ascontiguousarray(
        np.asarray(conv_b, dtype=np.float32).reshape(OUT // 128, 128).T)
    gc = np.full((128, 1), np.asarray(gamma, dtype=np.float32).reshape(-1)[0],
                 dtype=np.float32)

    in_maps = []
    for c in range(NCORES):
        qc = q[BL * c:BL * (c + 1)]              # [BL, C, HW]
        qtr = qc.transpose(0, 2, 1)              # [BL, HW, C]
        qT = np.zeros((BL, 2, 128, C), np.float32)
        qT[:, 0, :, :] = qtr[:, 0:128, :]
        qT[:, 1, 0:HW - 128, :] = qtr[:, 128:HW, :]
        qn = np.ascontiguousarray(qc.reshape(BL, CT, 128, HW))
        qb = np.ones((BL, CT, 128, HW + 1), ml_dtypes.bfloat16)
        qb[:, :, :, :HW] = qn.astype(ml_dtypes.bfloat16)
        in_maps.append({
            "qT": qT, "qn": qn, "qb": qb,
            "wT": wT, "bias": bias, "gammac": gc,
        })
    return in_maps


def run(x, gamma, conv_w, conv_b, trace=False, **kwargs):
    from concourse.bass_utils import run_bass_kernel_spmd

    nc = _get_nc()
    in_maps = _prep_in_maps(x, gamma, conv_w, conv_b)
    res = run_bass_kernel_spmd(nc, in_maps, core_ids=list(range(NCORES)),
                               trace=trace, **kwargs)
    outs = [np.asarray(res.results[i]["out"], dtype=np.float32)
            for i in range(NCORES)]
    full = np.concatenate(outs, axis=0).reshape(B, OUT, 14, 14)
    return full, res


def kernel(x, gamma, conv_w, conv_b):
    full, _ = run(x, gamma, conv_w, conv_b, trace=False)
    return full


# revision 14
# speedup vs baseline: 1.1139x; 1.1139x over previous
"""CAM (channel attention) module kernel for 8 TRN2 NeuronCores.

Reference computation (per batch b of 32, C=2048, N=H*W=196):
    E = q @ q.T                      # [C, C] channel energy
    A = softmax(rowmax(E) - E)       # == softmax(-E) row-wise (shift cancels)
    out = gamma * (A @ q) + x
    y = conv1x1(out, W) + bias       # 2048 -> 512

Sharding: pure data-parallel over batch, 4 batches per core, conv weights
replicated. No collectives.

Per-core kernel design (V2, symmetric-E):
  - E = q q^T is symmetric: compute only block-upper-triangle (row-tile j
    covers columns [256*(j//2), 2048) in 256-wide fp32r matmuls).
  - U = exp(-E - 40) via ScalarE activation straight out of PSUM into bf16
    (constant shift keeps exp in fp32 range for N(0,1) inputs of this size;
    any constant cancels in the softmax normalization).
  - Lower-triangle U blocks are mirror-filled with SBUF->SBUF DMA
    transposes (bf16 XBAR path, off-engine).
  - U symmetric => stored U tiles serve directly as lhsT for O = U @ q
    (bf16, free dim 197: q plus a ones column that yields the row sums Z).
  - normalize + residual: xout = O[:, :196] * (gamma/Z) + x on VectorE.
  - 1x1 conv as matmul over C with batch-pair-concatenated free dim (392)
    in float32r, bias added on the PSUM->SBUF evacuation.
"""

import numpy as np

B = 32
NCORES = 8
BL = B // NCORES  # batches per core
C = 2048
HW = 196
OUT = 512
CT = C // 128  # 16 c-tiles
SHIFT = 40.0

_CACHE = {}


def _build_nc(reps=1):
    import contextlib
    import concourse.bacc as bacc
    import concourse.tile as tile
    import concourse.mybir as mybir

    f32 = mybir.dt.float32
    f32r = mybir.dt.float32r
    bf16 = mybir.dt.bfloat16
    FT = mybir.ActivationFunctionType
    ALU = mybir.AluOpType

    nc = bacc.Bacc("TRN2", target_bir_lowering=False, debug=False,
                   num_devices=NCORES)

    qTd = nc.dram_tensor("qT", [BL, 2, 128, C], f32r, kind="ExternalInput")
    qnd = nc.dram_tensor("qn", [BL, CT, 128, HW], f32, kind="ExternalInput")
    qbd = nc.dram_tensor("qb", [BL, CT, 128, HW + 1], bf16,
                         kind="ExternalInput")
    wTd = nc.dram_tensor("wT", [CT, 128, OUT], f32r, kind="ExternalInput")
    biasd = nc.dram_tensor("bias", [128, OUT // 128], f32, kind="ExternalInput")
    gammad = nc.dram_tensor("gammac", [128, 1], f32, kind="ExternalInput")
    outd = nc.dram_tensor("out", [BL, OUT, HW], f32, kind="ExternalOutput")

    with tile.TileContext(nc) as tc:
        with (
            tc.tile_pool(name="const", bufs=1) as constp,
            tc.tile_pool(name="qt", bufs=2) as qtp,
            tc.tile_pool(name="qbp", bufs=2) as qbp,
            tc.tile_pool(name="qnp", bufs=2) as qnp,
            tc.tile_pool(name="u", bufs=1) as up,
            tc.tile_pool(name="xo", bufs=1) as xop,
            tc.tile_pool(name="y", bufs=4) as yp,
            tc.tile_pool(name="z", bufs=8) as zp,
            tc.tile_pool(name="psum", bufs=2, space="PSUM") as psp,
        ):
            wT_s = constp.tile([128, CT, OUT], f32r)
            nc.sync.dma_start(wT_s[:], wTd[:].rearrange("i p o -> p i o"))
            bias_s = constp.tile([128, OUT // 128], f32)
            nc.sync.dma_start(bias_s[:], biasd[:])
            gamma_s = constp.tile([128, 1], f32)
            nc.sync.dma_start(gamma_s[:], gammad[:])
            shift_s = constp.tile([128, 1], f32)
            nc.vector.memset(shift_s[:], -SHIFT)

            U = up.tile([128, CT, C], bf16)

            rep_ctx = (
                tc.For_i(0, reps, 1,
                         hint_engines=tuple(mybir.EngineType))
                if reps > 1 else contextlib.nullcontext()
            )
            with rep_ctx:
                xo = None
                for b in range(BL):
                    qT_s = qtp.tile([128, 2, C], f32r, tag="qt")
                    nc.sync.dma_start(qT_s[:], qTd[b].rearrange("k p c -> p k c"))
                    qb_s = qbp.tile([128, CT, HW + 1], bf16, tag="qb")
                    nc.sync.dma_start(qb_s[:], qbd[b].rearrange("i p n -> p i n"))
                    qn_s = qnp.tile([128, CT, HW], f32, tag="qn")
                    nc.sync.dma_start(qn_s[:], qnd[b].rearrange("i p n -> p i n"))

                    # ---- upper-triangle E (fp32r, 256-col chunks) + exp ----
                    for j in range(CT):
                        dstart = 256 * (j // 2)
                        pe = psp.tile([128, C], f32, tag="ps")
                        for m in range(dstart, C, 256):
                            for k in range(2):
                                nc.tensor.matmul(
                                    pe[:, m:m + 256],
                                    qT_s[:, k, 128 * j:128 * (j + 1)],
                                    qT_s[:, k, m:m + 256],
                                    start=(k == 0),
                                    stop=(k == 1),
                                )
                        nc.scalar.activation(
                            U[:, j, dstart:], pe[:, dstart:], FT.Exp,
                            bias=shift_s[:], scale=-1.0,
                        )

                    # ---- mirror lower-triangle blocks: one batched
                    # block-transpose DMA per source row-tile t writes
                    # U[:, j, 128t:128(t+1)] = U[t-tile block j]^T for all
                    # j >= j0(t) (3D-out XBAR transpose semantics).
                    for t in range(CT):
                        j0 = 2 * (t // 2) + 2
                        if j0 >= CT:
                            continue
                        nc.sync.dma_start_transpose(
                            U[:, j0:CT, 128 * t:128 * (t + 1)],
                            U[:, t, 128 * j0:C],
                        )

                    if b % 2 == 0:
                        xo = xop.tile([128, CT, 2 * HW], f32r, tag="xo")
                    off = (b % 2) * HW

                    # ---- O|Z = U @ [q|1]; high j first (they need no mirrors)
                    for j in range(CT - 1, -1, -1):
                        po = psp.tile([128, C], f32, tag="ps")
                        for i in range(CT):
                            nc.tensor.matmul(
                                po[:, :HW + 1],
                                U[:, i, 128 * j:128 * (j + 1)],
                                qb_s[:, i, :],
                                start=(i == 0),
                                stop=(i == CT - 1),
                            )
                        rg = zp.tile([128, 2], f32, tag="rg")
                        nc.vector.reciprocal(rg[:, 0:1], po[:, HW:HW + 1])
                        nc.vector.tensor_tensor(
                            rg[:, 1:2], rg[:, 0:1], gamma_s[:], ALU.mult)
                        nc.vector.tensor_scalar_mul(
                            xo[:, j, off:off + HW], po[:, :HW], rg[:, 1:2])
                        nc.vector.tensor_tensor(
                            xo[:, j, off:off + HW], xo[:, j, off:off + HW],
                            qn_s[:, j, :], ALU.add)

                    # ---- 1x1 conv on a pair of batches (free dim 392) ----
                    if b % 2 == 1:
                        b0 = b - 1
                        for t in range(OUT // 128):
                            pc = psp.tile([128, C], f32, tag="ps")
                            for i in range(CT):
                                nc.tensor.matmul(
                                    pc[:, :2 * HW],
                                    wT_s[:, i, 128 * t:128 * (t + 1)],
                                    xo[:, i, :],
                                    start=(i == 0),
                                    stop=(i == CT - 1),
                                )
                            y = yp.tile([128, 2, HW], f32, tag="y")
                            nc.vector.tensor_scalar_add(
                                y[:], pc[:, :2 * HW], bias_s[:, t:t + 1])
                            nc.sync.dma_start(
                                outd[b0:b0 + 2, 128 * t:128 * (t + 1), :]
                                .rearrange("b p n -> p b n"),
                                y[:])

    nc.compile()
    return nc


def _get_nc():
    if "nc" not in _CACHE:
        _CACHE["nc"] = _build_nc()
    return _CACHE["nc"]


def _prep_in_maps(x, gamma, conv_w, conv_b):
    import ml_dtypes

    x = np.ascontiguousarray(np.asarray(x, dtype=np.float32))
    q = x.reshape(B, C, HW)
    W2 = np.asarray(conv_w, dtype=np.float32).reshape(OUT, C)
    wT = np.ascontiguousarray(W2.T).reshape(CT, 128, OUT)
    bias = np.ascontiguousarray(
        np.asarray(conv_b, dtype=np.float32).reshape(OUT // 128, 128).T)
    gc = np.full((128, 1), np.asarray(gamma, dtype=np.float32).reshape(-1)[0],
                 dtype=np.float32)

    in_maps = []
    for c in range(NCORES):
        qc = q[BL * c:BL * (c + 1)]              # [BL, C, HW]
        qtr = qc.transpose(0, 2, 1)              # [BL, HW, C]
        qT = np.zeros((BL, 2, 128, C), np.float32)
        qT[:, 0, :, :] = qtr[:, 0:128, :]
        qT[:, 1, 0:HW - 128, :] = qtr[:, 128:HW, :]
        qn = np.ascontiguousarray(qc.reshape(BL, CT, 128, HW))
        qb = np.ones((BL, CT, 128, HW + 1), ml_dtypes.bfloat16)
        qb[:, :, :, :HW] = qn.astype(ml_dtypes.bfloat16)
        in_maps.append({
            "qT": qT, "qn": qn, "qb": qb,
            "wT": wT, "bias": bias, "gammac": gc,
        })
    return in_maps


def run(x, gamma, conv_w, conv_b, trace=False, **kwargs):
    from concourse.bass_utils import run_bass_kernel_spmd

    nc = _get_nc()
    in_maps = _prep_in_maps(x, gamma, conv_w, conv_b)
    res = run_bass_kernel_spmd(nc, in_maps, core_ids=list(range(NCORES)),
                               trace=trace, **kwargs)
    outs = [np.asarray(res.results[i]["out"], dtype=np.float32)
            for i in range(NCORES)]
    full = np.concatenate(outs, axis=0).reshape(B, OUT, 14, 14)
    return full, res


def kernel(x, gamma, conv_w, conv_b):
    full, _ = run(x, gamma, conv_w, conv_b, trace=False)
    return full
